# revision 18
# baseline (speedup 1.0000x reference)
"""Trainium2 Bass kernel for a PointNet++-style feature-propagation decoder
(4 stages of kNN(k=3) inverse-distance-weighted feature interpolation).

Sharding: batch b = core//2 (data parallel over B=4), and the finest stage's
8192 query points split in half across each core pair (point parallel along N
per the sharding hint). Stages 0-2 are duplicated within a pair (cheap);
stage 3 dominates and is n-split. Output rows 0:64 are the x0 passthrough,
assembled on the host.

Per-core device pipeline per stage:
  - negated squared distances via one K=5 PE matmul per 128-query tile:
      A = [ax, ay, az, -|a|^2, -1], B = [2bx, 2by, 2bz, 1, |b|^2], A.B = -dist
  - top-3 neighbors via DVE max (top-8) + max_index
  - inverse-distance weights on DVE
  - feature gather via SWDGE indirect DMA (row gather from a DRAM source
    table, one row per partition), weighted 3-way combine via
    scalar_tensor_tensor (per-partition scalar FMA)
  - stage output rows are DMA'd into the next stage's DRAM source table;
    the final stage is transposed back to [D, N] layout via PE transposes.
"""

import numpy as np

P = 128
KNN = 3
EPS = 1e-8

B = 4
NS = [8192, 2048, 512, 128, 32]  # points per level, finest -> coarsest
CS = [64, 128, 256, 512, 1024]   # feature channels per level

_CACHED = {"nc": None, "key": None}


def _build_program(ns, cs, n_half, split_waits=True):
    """Trace the per-core Bass program. ns/cs as in reference (finest first).
    n_half: number of finest-level query points this core handles."""
    import contextlib

    import concourse.bass as bass
    import concourse.mybir as mybir
    import concourse.tile as tile
    from concourse.bass import IndirectOffsetOnAxis
    from concourse.masks import make_identity

    _patch_tile_drain()

    f32 = mybir.dt.float32
    u32 = mybir.dt.uint32
    Alu = mybir.AluOpType
    Axis = mybir.AxisListType

    # stage s: fine level (3-s) [half of level 0 for s=3], coarse level (4-s)
    # d2[s]: width of the source table gathered at stage s
    d2 = [cs[4]]
    for s in range(1, 4):
        d2.append(cs[4 - s] + d2[s - 1])
    stages = []
    for s in range(4):
        nf = ns[3 - s] if s < 3 else n_half
        stages.append(dict(nf=nf, S=ns[4 - s], d2=d2[s],
                           cx=(cs[3 - s] if s < 3 else None)))

    nc = bass.Bass("TRN2")

    # ---- external inputs (per core) ----
    xt = {}   # xyz transposed [3, N]
    xr = {}   # xyz rows [N, 3]
    for i, n in enumerate(ns):
        nn_ = n_half if i == 0 else n
        xt[i] = nc.dram_tensor(f"xt{i}", [3, nn_], f32, kind="ExternalInput")
        xr[i] = nc.dram_tensor(f"xr{i}", [nn_, 3], f32, kind="ExternalInput")
    feat = {}
    for i in range(1, 5):
        feat[i] = nc.dram_tensor(f"f{i}", [cs[i], ns[i]], f32, kind="ExternalInput")

    # ---- external output: interp part of final stage, [d2[3], n_half] ----
    oi = nc.dram_tensor("oi", [d2[3], n_half], f32, kind="ExternalOutput")

    # ---- internal DRAM source tables ----
    tsrc = []
    for s in range(4):
        tsrc.append(nc.dram_tensor(f"tsrc{s}", [stages[s]["S"], stages[s]["d2"]], f32))
    # scratch for reshaping row-layout norms into free-layout matmul rows
    nscr = {s: (nc.dram_tensor(f"nscrf{s}", [stages[s]["nf"]], f32),
                nc.dram_tensor(f"nscrc{s}", [max(stages[s]["S"], P)], f32))
            for s in range(4)}

    with tile.TileContext(nc) as tc, contextlib.ExitStack() as ctx:
        cpool = ctx.enter_context(tc.tile_pool(name="const", bufs=1))
        inpool = ctx.enter_context(tc.tile_pool(name="in", bufs=1))
        abpool = ctx.enter_context(tc.tile_pool(name="ab", bufs=1))
        ndpool = ctx.enter_context(tc.tile_pool(name="nd", bufs=3))
        smpool = ctx.enter_context(tc.tile_pool(name="sm", bufs=2))
        gpool = ctx.enter_context(tc.tile_pool(name="g", bufs=2))
        rpool = ctx.enter_context(tc.tile_pool(name="r", bufs=2))
        xtp = ctx.enter_context(tc.tile_pool(name="xtp", bufs=2))
        ps_nd = ctx.enter_context(tc.tile_pool(name="ps_nd", bufs=2, space="PSUM"))
        ps_tp = ctx.enter_context(tc.tile_pool(name="ps_tp", bufs=4, space="PSUM"))

        ident = cpool.tile([P, P], f32, tag="ident")
        make_identity(nc, ident[:])
        # constant rows (engines can't start at odd partitions; build at
        # partition 0 and DMA into place)
        maxn = max(st["nf"] for st in stages)
        cones = cpool.tile([1, maxn], f32, tag="cones")
        nc.vector.memset(cones[:], 1.0)
        cneg = cpool.tile([1, maxn], f32, tag="cneg")
        nc.vector.memset(cneg[:], -1.0)

        # ---------- build x^T parts of the source tables ----------
        def xt_into_table(fi, table, nrows, ncols):
            """write feat[fi]^T ([nrows points, ncols feats]) into
            table[:, 0:ncols] via PE transposes."""
            cchunks = (ncols + P - 1) // P
            rchunks = (nrows + P - 1) // P
            fsb = inpool.tile([min(ncols, P), cchunks * nrows], f32,
                              tag="fsb")
            src = feat[fi][:, :]
            if ncols > P:
                nc.sync.dma_start(
                    fsb[:].rearrange("p (cc n) -> p cc n", cc=cchunks),
                    src.rearrange("(cc p) n -> p cc n", p=P))
            else:
                nc.sync.dma_start(fsb[:ncols, :nrows], src)
            for t in range(rchunks):
                rows = min(P, nrows - t * P)
                xtt = xtp.tile([P, cchunks * P], f32, tag="xtt")
                for cc in range(cchunks):
                    c0 = cc * P
                    cw = min(P, ncols - c0)
                    pst = ps_tp.tile([P, P], f32, tag="pst")
                    nc.tensor.transpose(
                        pst[:rows, :cw],
                        fsb[:cw, cc * nrows + t * P: cc * nrows + t * P + rows],
                        ident[:])
                    nc.scalar.copy(xtt[:rows, c0:c0 + cw], pst[:rows, :cw])
                nc.sync.dma_start(table[t * P:t * P + rows, 0:ncols],
                                  xtt[:rows, :ncols])

        xt_into_table(4, tsrc[0], stages[0]["S"], cs[4])   # x4^T -> T0
        xt_into_table(3, tsrc[1], stages[1]["S"], cs[3])   # x3^T -> T1
        xt_into_table(2, tsrc[2], stages[2]["S"], cs[2])   # x2^T -> T2
        xt_into_table(1, tsrc[3], stages[3]["S"], cs[1])   # x1^T -> T3

        # ---------- row-layout squared norms ----------
        def row_norms(xr_dram, n, negate, tag):
            """[P, T] tile holding (+-)|p|^2 of n points, point (t*128+p) at
            [p, t] (or [p, 0] for p < n when n < 128)."""
            if n >= P:
                T = n // P
                rx = smpool.tile([P, T * 3], f32, tag=f"rx{tag}")
                nc.sync.dma_start(
                    rx[:].rearrange("p (t c) -> p t c", c=3),
                    xr_dram[:, :].rearrange("(t p) c -> p t c", p=P))
                pdim = P
            else:
                T = 1
                rx = smpool.tile([P, 3], f32, tag=f"rx{tag}")
                nc.sync.dma_start(rx[:n, :], xr_dram[:, :])
                pdim = n
            sq = smpool.tile([P, T * 3], f32, tag=f"sq{tag}")
            nc.vector.tensor_tensor(sq[:pdim, :T * 3], rx[:pdim, :T * 3],
                                    rx[:pdim, :T * 3], op=Alu.mult)
            nrm = smpool.tile([P, T], f32, tag=f"nrm{tag}")
            nc.vector.tensor_reduce(
                nrm[:pdim, :T],
                sq[:pdim, :T * 3].rearrange("p (t c) -> p t c", c=3),
                axis=Axis.X, op=Alu.add)
            if negate:
                nc.vector.tensor_scalar_mul(nrm[:pdim, :T], nrm[:pdim, :T], -1.0)
            return nrm, T

        # ---------- stages ----------
        for s, st in enumerate(stages):
            nf, S, D2 = st["nf"], st["S"], st["d2"]
            T = nf // P
            fine_lvl = 3 - s if s < 3 else 0
            coarse_lvl = 4 - s

            # A5 [5, nf]: rows 0-2 xt_fine, row3 -|a|^2, row4 -1
            a5 = abpool.tile([5, nf], f32, tag="a5")
            nc.sync.dma_start(a5[0:3, :], xt[fine_lvl][:, :])
            nfin, Tf = row_norms(xr[fine_lvl], nf, negate=True, tag="f")
            nc.sync.dma_start(
                nscr[s][0][:].rearrange("(t j) -> j t", j=P), nfin[:, :Tf])
            nc.sync.dma_start(a5[3:4, :], nscr[s][0][:])
            nc.sync.dma_start(a5[4:5, :], cneg[:, :nf])

            # B5 [5, S]: rows 0-2 2*xt_coarse, row3 1, row4 +|b|^2
            b5 = abpool.tile([5, S], f32, tag="b5")
            nc.sync.dma_start(b5[0:3, :], xt[coarse_lvl][:, :])
            nc.vector.tensor_scalar_mul(b5[0:3, :], b5[0:3, :], 2.0)
            nc.sync.dma_start(b5[3:4, :], cones[:, :S])
            ncr, Tc = row_norms(xr[coarse_lvl], S, negate=False, tag="c")
            if S >= P:
                nc.sync.dma_start(
                    nscr[s][1][:S].rearrange("(t j) -> j t", j=P),
                    ncr[:, :Tc])
                nc.sync.dma_start(b5[4:5, :], nscr[s][1][:S])
            else:
                nc.sync.dma_start(nscr[s][1][:S], ncr[:S, :1])
                nc.sync.dma_start(b5[4:5, :], nscr[s][1][:S])

            # ---- distances + top-3 per 128-query tile ----
            maxb = smpool.tile([P, T * 8], f32, tag=f"maxb{s}")
            idxb = smpool.tile([P, T * 8], u32, tag=f"idxb{s}")
            nchunk = (S + 511) // 512
            for t in range(T):
                nd_sb = ndpool.tile([P, S], f32, tag="nd_sb")
                pnd = ps_nd.tile([P, min(S, 1024)], f32, tag="pnd")
                for c in range(nchunk):
                    w = min(512, S - c * 512)
                    nc.tensor.matmul(
                        pnd[:, (c % 2) * 512:(c % 2) * 512 + w],
                        a5[:, t * P:(t + 1) * P],
                        b5[:, c * 512:c * 512 + w],
                        start=True, stop=True)
                    if c % 2 == 1 or c == nchunk - 1:
                        lo = (c // 2) * 1024
                        w2 = min(1024, S - lo)
                        nc.scalar.copy(nd_sb[:, lo:lo + w2], pnd[:, :w2])
                        if c != nchunk - 1:
                            pnd = ps_nd.tile([P, min(S, 1024)], f32, tag="pnd")
                nc.vector.max(maxb[:, t * 8:(t + 1) * 8], nd_sb[:])
                nc.vector.max_index(idxb[:, t * 8:(t + 1) * 8],
                                    maxb[:, t * 8:(t + 1) * 8], nd_sb[:])

            # ---- weights: w_k = (1/(d_k+eps)) / sum_j (1/(d_j+eps)) ----
            top3 = maxb[:].rearrange("p (t e) -> p t e", e=8)[:, :, 0:KNN]
            dbuf = smpool.tile([P, T * KNN], f32, tag=f"dbuf{s}")
            nc.vector.tensor_scalar(dbuf[:].rearrange("p (t e) -> p t e", e=KNN),
                                    top3, -1.0, EPS, op0=Alu.mult, op1=Alu.add)
            wraw = smpool.tile([P, T * KNN], f32, tag=f"wraw{s}")
            nc.vector.reciprocal(wraw[:], dbuf[:])
            wsum = smpool.tile([P, T], f32, tag=f"wsum{s}")
            nc.vector.tensor_reduce(
                wsum[:], wraw[:].rearrange("p (t e) -> p t e", e=KNN),
                axis=Axis.X, op=Alu.add)
            wnrm = smpool.tile([P, T], f32, tag=f"wnrm{s}")
            nc.vector.reciprocal(wnrm[:], wsum[:])
            wgt = smpool.tile([P, T * KNN], f32, tag=f"wgt{s}")
            nc.vector.tensor_tensor(
                wgt[:].rearrange("p (t e) -> p t e", e=KNN),
                wraw[:].rearrange("p (t e) -> p t e", e=KNN),
                wnrm[:].rearrange("p (t o) -> p t o", o=1).to_broadcast([P, T, KNN]),
                op=Alu.mult)

            # ---- gather + weighted combine per tile ----
            for t in range(T):
                gts = []
                for k in range(KNN):
                    gt = gpool.tile([P, D2], f32, tag=f"g{k}")
                    nc.gpsimd.indirect_dma_start(
                        out=gt[:], out_offset=None,
                        in_=tsrc[s][:, :],
                        in_offset=IndirectOffsetOnAxis(
                            ap=idxb[:, t * 8 + k:t * 8 + k + 1], axis=0))
                    gts.append(gt)
                ra = rpool.tile([P, D2], f32, tag="ra")
                rb = rpool.tile([P, D2], f32, tag="rb")
                nc.vector.tensor_scalar_mul(ra[:], gts[0][:],
                                            wgt[:, t * 3:t * 3 + 1])
                nc.vector.scalar_tensor_tensor(
                    rb[:], gts[1][:], wgt[:, t * 3 + 1:t * 3 + 2], ra[:],
                    op0=Alu.mult, op1=Alu.add)
                nc.vector.scalar_tensor_tensor(
                    ra[:], gts[2][:], wgt[:, t * 3 + 2:t * 3 + 3], rb[:],
                    op0=Alu.mult, op1=Alu.add)

                if s < 3:
                    nc.sync.dma_start(
                        tsrc[s + 1][t * P:(t + 1) * P, st["cx"]:st["cx"] + D2],
                        ra[:])
                else:
                    # transpose [128, D2] to column layout and DMA to oi
                    dchunks = D2 // P
                    colb = rpool.tile([P, D2], f32, tag="colb")
                    for dd in range(dchunks):
                        pst = ps_tp.tile([P, P], f32, tag="pst")
                        nc.tensor.transpose(
                            pst[:], ra[:, dd * P:(dd + 1) * P], ident[:])
                        nc.scalar.copy(colb[:, dd * P:(dd + 1) * P], pst[:])
                    nc.sync.dma_start(
                        oi.rearrange("(dd p) (t j) -> p dd t j",
                                     p=P, j=P)[:, :, t, :],
                        colb[:].rearrange("p (dd j) -> p dd j", j=P))
    if split_waits:
        _split_multi_waits(nc)
    return nc


def _split_multi_waits(nc):
    """This walrus build rejects instructions carrying more than one sync
    wait. Hoist extra waits into same-engine NoOps inserted just before."""
    import concourse.mybir as mybir

    n = 0
    for f in nc.m.functions:
        for bb in f.blocks:
            il = bb.instructions
            i = 0
            while i < len(il):
                inst = il[i]
                si = getattr(inst, "sync_info", None)
                ow = list(si.on_wait) if si is not None else []
                if len(ow) > 1:
                    for w in ow[:-1]:
                        nop = mybir.InstNoOp(name=f"W{n}-{inst.name}",
                                             ins=[], outs=[])
                        n += 1
                        nop.engine = inst.engine
                        nop.sync_info = mybir.SyncInfo(on_update=[],
                                                       on_wait=[w])
                        il.insert(i, nop)
                        i += 1
                    inst.sync_info = mybir.SyncInfo(
                        on_update=list(si.on_update), on_wait=[ow[-1]])
                i += 1


def _patch_tile_drain():
    """This walrus build rejects >1 sync-wait on the kernel-tail Drain; spread
    the waits across single-wait SP nops instead."""
    import concourse.mybir as mybir
    import concourse.tile as tile
    from concourse.vector_clock import ScopedClock

    if getattr(tile.TileContext, "_drain_patched", False):
        return

    def _patched(self, tick_clock, wait_clock):
        nc = self.nc
        probe = nc.sync.nop()
        wait_clock.add_sem_waits(probe.ins,
                                 ScopedClock({None: tick_clock.global_clock}))
        si = probe.ins.sync_info
        ow = list(si.on_wait) if si is not None else []
        if len(ow) > 1:
            for w in ow[1:]:
                n2 = nc.sync.nop()
                n2.ins.sync_info = mybir.SyncInfo(on_update=[], on_wait=[w])
            probe.ins.sync_info = mybir.SyncInfo(on_update=list(si.on_update),
                                                 on_wait=[ow[0]])
        nc.sync.drain()
        nc.all_engine_barrier()
        assert self.sems is not None
        popped = nc._tile_sem_poison_stack.pop()
        assert popped is self._sem_poison
        nc.clear_and_free_semaphores(list(self.sems.allocated().values()))
        nc.all_engine_barrier()

    tile.TileContext._drain_and_barrier = _patched
    tile.TileContext._drain_patched = True


def _get_program(ns, cs, n_half):
    key = (tuple(ns), tuple(cs), n_half)
    if _CACHED["key"] != key:
        _CACHED["nc"] = _build_program(ns, cs, n_half)
        _CACHED["key"] = key
    return _CACHED["nc"]


def make_core_inputs(inputs, ns, n_half, core):
    """Slice/transform full inputs for one core (b = core//2, half = core%2)."""
    b, h = core // 2, core % 2
    d = {}
    x0h = np.ascontiguousarray(np.asarray(inputs["xyz0"])[b, h * n_half:(h + 1) * n_half])
    d["xt0"] = np.ascontiguousarray(x0h.T)
    d["xr0"] = x0h
    for i in range(1, 5):
        xi = np.ascontiguousarray(np.asarray(inputs[f"xyz{i}"])[b])
        d[f"xt{i}"] = np.ascontiguousarray(xi.T)
        d[f"xr{i}"] = xi
        d[f"f{i}"] = np.ascontiguousarray(np.asarray(inputs[f"x{i}"])[b])
    return d


def kernel(**inputs):
    from concourse.bass_utils import run_bass_kernel_spmd

    ns, cs = NS, CS
    n_half = ns[0] // 2
    nc = _get_program(ns, cs, n_half)

    in_maps = [make_core_inputs(inputs, ns, n_half, c) for c in range(8)]
    res = run_bass_kernel_spmd(nc, in_maps, core_ids=list(range(8)))

    dout = sum(cs)
    out = np.empty((B, dout, ns[0]), np.float32)
    out[:, :cs[0], :] = np.asarray(inputs["x0"])
    for c in range(8):
        b, h = c // 2, c % 2
        out[b, cs[0]:, h * n_half:(h + 1) * n_half] = res.results[c]["oi"]
    return out


# revision 19
# speedup vs baseline: 1.1383x; 1.1383x over previous
"""Trainium2 Bass kernel for a PointNet++-style feature-propagation decoder
(4 stages of kNN(k=3) inverse-distance-weighted feature interpolation).

Sharding: batch b = core//2 (data parallel over B=4), and the finest stage's
8192 query points split in half across each core pair (point parallel along N
per the sharding hint). Stages 0-2 are duplicated within a pair (cheap);
stage 3 dominates and is n-split. Output rows 0:64 are the x0 passthrough,
assembled on the host.

Per-core device pipeline per stage:
  - negated squared distances via one K=5 PE matmul per 128-query tile:
      A = [ax, ay, az, -|a|^2, -1], B = [2bx, 2by, 2bz, 1, |b|^2], A.B = -dist
  - top-3 neighbors via DVE max (top-8) + max_index
  - inverse-distance weights on DVE
  - feature gather via SWDGE indirect DMA (row gather from a DRAM source
    table, one row per partition), weighted 3-way combine via
    scalar_tensor_tensor (per-partition scalar FMA)
  - stage output rows are DMA'd into the next stage's DRAM source table;
    the final stage is transposed back to [D, N] layout via PE transposes.
"""

import numpy as np

P = 128
KNN = 3
EPS = 1e-8

B = 4
NS = [8192, 2048, 512, 128, 32]  # points per level, finest -> coarsest
CS = [64, 128, 256, 512, 1024]   # feature channels per level

_CACHED = {"nc": None, "key": None}


def _build_program(ns, cs, n_half, split_waits=True):
    """Trace the per-core Bass program. ns/cs as in reference (finest first).
    n_half: number of finest-level query points this core handles."""
    import contextlib

    import concourse.bass as bass
    import concourse.mybir as mybir
    import concourse.tile as tile
    from concourse.bass import IndirectOffsetOnAxis
    from concourse.masks import make_identity

    _patch_tile_drain()

    f32 = mybir.dt.float32
    u32 = mybir.dt.uint32
    Alu = mybir.AluOpType
    Axis = mybir.AxisListType

    # stage s: fine level (3-s) [half of level 0 for s=3], coarse level (4-s)
    # d2[s]: width of the source table gathered at stage s
    d2 = [cs[4]]
    for s in range(1, 4):
        d2.append(cs[4 - s] + d2[s - 1])
    stages = []
    for s in range(4):
        nf = ns[3 - s] if s < 3 else n_half
        stages.append(dict(nf=nf, S=ns[4 - s], d2=d2[s],
                           cx=(cs[3 - s] if s < 3 else None)))

    nc = bass.Bass("TRN2")

    # ---- external inputs (per core) ----
    xt = {}   # xyz transposed [3, N]
    xr = {}   # xyz rows [N, 3]
    for i, n in enumerate(ns):
        nn_ = n_half if i == 0 else n
        xt[i] = nc.dram_tensor(f"xt{i}", [3, nn_], f32, kind="ExternalInput")
        xr[i] = nc.dram_tensor(f"xr{i}", [nn_, 3], f32, kind="ExternalInput")
    feat = {}
    for i in range(1, 5):
        feat[i] = nc.dram_tensor(f"f{i}", [cs[i], ns[i]], f32, kind="ExternalInput")

    # ---- external output: interp part of final stage, [d2[3], n_half] ----
    oi = nc.dram_tensor("oi", [d2[3], n_half], f32, kind="ExternalOutput")

    # ---- internal DRAM source tables ----
    tsrc = []
    for s in range(4):
        tsrc.append(nc.dram_tensor(f"tsrc{s}", [stages[s]["S"], stages[s]["d2"]], f32))
    # scratch for reshaping row-layout norms into free-layout matmul rows
    nscr = {s: (nc.dram_tensor(f"nscrf{s}", [stages[s]["nf"]], f32),
                nc.dram_tensor(f"nscrc{s}", [max(stages[s]["S"], P)], f32))
            for s in range(4)}

    with tile.TileContext(nc) as tc, contextlib.ExitStack() as ctx:
        cpool = ctx.enter_context(tc.tile_pool(name="const", bufs=1))
        inpool = ctx.enter_context(tc.tile_pool(name="in", bufs=1))
        abpool = ctx.enter_context(tc.tile_pool(name="ab", bufs=1))
        ndpool = ctx.enter_context(tc.tile_pool(name="nd", bufs=3))
        smpool = ctx.enter_context(tc.tile_pool(name="sm", bufs=2))
        gpool = ctx.enter_context(tc.tile_pool(name="g", bufs=2))
        rpool = ctx.enter_context(tc.tile_pool(name="r", bufs=2))
        xtp = ctx.enter_context(tc.tile_pool(name="xtp", bufs=2))
        ps_nd = ctx.enter_context(tc.tile_pool(name="ps_nd", bufs=2, space="PSUM"))
        ps_tp = ctx.enter_context(tc.tile_pool(name="ps_tp", bufs=4, space="PSUM"))

        ident = cpool.tile([P, P], f32, tag="ident")
        make_identity(nc, ident[:])
        # constant rows (engines can't start at odd partitions; build at
        # partition 0 and DMA into place)
        maxn = max(st["nf"] for st in stages)
        cones = cpool.tile([1, maxn], f32, tag="cones")
        nc.vector.memset(cones[:], 1.0)
        cneg = cpool.tile([1, maxn], f32, tag="cneg")
        nc.vector.memset(cneg[:], -1.0)

        # ---------- build x^T parts of the source tables ----------
        def xt_into_table(fi, table, nrows, ncols):
            """write feat[fi]^T ([nrows points, ncols feats]) into
            table[:, 0:ncols] via PE transposes."""
            cchunks = (ncols + P - 1) // P
            rchunks = (nrows + P - 1) // P
            fsb = inpool.tile([min(ncols, P), cchunks * nrows], f32,
                              tag="fsb")
            src = feat[fi][:, :]
            if ncols > P:
                nc.sync.dma_start(
                    fsb[:].rearrange("p (cc n) -> p cc n", cc=cchunks),
                    src.rearrange("(cc p) n -> p cc n", p=P))
            else:
                nc.sync.dma_start(fsb[:ncols, :nrows], src)
            for t in range(rchunks):
                rows = min(P, nrows - t * P)
                xtt = xtp.tile([P, cchunks * P], f32, tag="xtt")
                for cc in range(cchunks):
                    c0 = cc * P
                    cw = min(P, ncols - c0)
                    pst = ps_tp.tile([P, P], f32, tag="pst")
                    nc.tensor.transpose(
                        pst[:rows, :cw],
                        fsb[:cw, cc * nrows + t * P: cc * nrows + t * P + rows],
                        ident[:])
                    nc.scalar.copy(xtt[:rows, c0:c0 + cw], pst[:rows, :cw])
                nc.sync.dma_start(table[t * P:t * P + rows, 0:ncols],
                                  xtt[:rows, :ncols])

        xt_into_table(4, tsrc[0], stages[0]["S"], cs[4])   # x4^T -> T0
        xt_into_table(3, tsrc[1], stages[1]["S"], cs[3])   # x3^T -> T1
        xt_into_table(2, tsrc[2], stages[2]["S"], cs[2])   # x2^T -> T2
        xt_into_table(1, tsrc[3], stages[3]["S"], cs[1])   # x1^T -> T3

        # ---------- row-layout squared norms ----------
        def row_norms(xr_dram, n, negate, tag):
            """[P, T] tile holding (+-)|p|^2 of n points, point (t*128+p) at
            [p, t] (or [p, 0] for p < n when n < 128)."""
            if n >= P:
                T = n // P
                rx = smpool.tile([P, T * 3], f32, tag=f"rx{tag}")
                nc.sync.dma_start(
                    rx[:].rearrange("p (t c) -> p t c", c=3),
                    xr_dram[:, :].rearrange("(t p) c -> p t c", p=P))
                pdim = P
            else:
                T = 1
                rx = smpool.tile([P, 3], f32, tag=f"rx{tag}")
                nc.sync.dma_start(rx[:n, :], xr_dram[:, :])
                pdim = n
            sq = smpool.tile([P, T * 3], f32, tag=f"sq{tag}")
            nc.vector.tensor_tensor(sq[:pdim, :T * 3], rx[:pdim, :T * 3],
                                    rx[:pdim, :T * 3], op=Alu.mult)
            nrm = smpool.tile([P, T], f32, tag=f"nrm{tag}")
            nc.vector.tensor_reduce(
                nrm[:pdim, :T],
                sq[:pdim, :T * 3].rearrange("p (t c) -> p t c", c=3),
                axis=Axis.X, op=Alu.add)
            if negate:
                nc.vector.tensor_scalar_mul(nrm[:pdim, :T], nrm[:pdim, :T], -1.0)
            return nrm, T

        # ---------- stages ----------
        for s, st in enumerate(stages):
            nf, S, D2 = st["nf"], st["S"], st["d2"]
            T = nf // P
            fine_lvl = 3 - s if s < 3 else 0
            coarse_lvl = 4 - s

            # A5 [5, nf]: rows 0-2 xt_fine, row3 -|a|^2, row4 -1
            a5 = abpool.tile([5, nf], f32, tag="a5")
            nc.sync.dma_start(a5[0:3, :], xt[fine_lvl][:, :])
            nfin, Tf = row_norms(xr[fine_lvl], nf, negate=True, tag="f")
            nc.sync.dma_start(
                nscr[s][0][:].rearrange("(t j) -> j t", j=P), nfin[:, :Tf])
            nc.sync.dma_start(a5[3:4, :], nscr[s][0][:])
            nc.sync.dma_start(a5[4:5, :], cneg[:, :nf])

            # B5 [5, S]: rows 0-2 2*xt_coarse, row3 1, row4 +|b|^2
            b5 = abpool.tile([5, S], f32, tag="b5")
            nc.sync.dma_start(b5[0:3, :], xt[coarse_lvl][:, :])
            nc.vector.tensor_scalar_mul(b5[0:3, :], b5[0:3, :], 2.0)
            nc.sync.dma_start(b5[3:4, :], cones[:, :S])
            ncr, Tc = row_norms(xr[coarse_lvl], S, negate=False, tag="c")
            if S >= P:
                nc.sync.dma_start(
                    nscr[s][1][:S].rearrange("(t j) -> j t", j=P),
                    ncr[:, :Tc])
                nc.sync.dma_start(b5[4:5, :], nscr[s][1][:S])
            else:
                nc.sync.dma_start(nscr[s][1][:S], ncr[:S, :1])
                nc.sync.dma_start(b5[4:5, :], nscr[s][1][:S])

            # ---- block-pipelined dist+top3 / weights / gather+combine ----
            maxb = smpool.tile([P, T * 8], f32, tag=f"maxb{s}")
            idxb = smpool.tile([P, T * 8], u32, tag=f"idxb{s}")
            dbuf = smpool.tile([P, T * KNN], f32, tag=f"dbuf{s}")
            wraw = smpool.tile([P, T * KNN], f32, tag=f"wraw{s}")
            wsum = smpool.tile([P, T], f32, tag=f"wsum{s}")
            wnrm = smpool.tile([P, T], f32, tag=f"wnrm{s}")
            wgt = smpool.tile([P, T * KNN], f32, tag=f"wgt{s}")
            nchunk = (S + 511) // 512
            BLK = 8
            for b0 in range(0, T, BLK):
              bn = min(BLK, T - b0)
              for t in range(b0, b0 + bn):
                nd_sb = ndpool.tile([P, S], f32, tag="nd_sb")
                pnd = ps_nd.tile([P, min(S, 1024)], f32, tag="pnd")
                for c in range(nchunk):
                    w = min(512, S - c * 512)
                    nc.tensor.matmul(
                        pnd[:, (c % 2) * 512:(c % 2) * 512 + w],
                        a5[:, t * P:(t + 1) * P],
                        b5[:, c * 512:c * 512 + w],
                        start=True, stop=True)
                    if c % 2 == 1 or c == nchunk - 1:
                        lo = (c // 2) * 1024
                        w2 = min(1024, S - lo)
                        nc.scalar.copy(nd_sb[:, lo:lo + w2], pnd[:, :w2])
                        if c != nchunk - 1:
                            pnd = ps_nd.tile([P, min(S, 1024)], f32, tag="pnd")
                nc.vector.max(maxb[:, t * 8:(t + 1) * 8], nd_sb[:])
                nc.vector.max_index(idxb[:, t * 8:(t + 1) * 8],
                                    maxb[:, t * 8:(t + 1) * 8], nd_sb[:])

              # weights for this block
              top3 = maxb[:, b0 * 8:(b0 + bn) * 8].rearrange(
                  "p (t e) -> p t e", e=8)[:, :, 0:KNN]
              d3 = dbuf[:, b0 * KNN:(b0 + bn) * KNN]
              w3 = wraw[:, b0 * KNN:(b0 + bn) * KNN]
              g3 = wgt[:, b0 * KNN:(b0 + bn) * KNN]
              nc.vector.tensor_scalar(d3.rearrange("p (t e) -> p t e", e=KNN),
                                      top3, -1.0, EPS, op0=Alu.mult, op1=Alu.add)
              nc.vector.reciprocal(w3, d3)
              nc.vector.tensor_reduce(
                  wsum[:, b0:b0 + bn], w3.rearrange("p (t e) -> p t e", e=KNN),
                  axis=Axis.X, op=Alu.add)
              nc.vector.reciprocal(wnrm[:, b0:b0 + bn], wsum[:, b0:b0 + bn])
              nc.vector.tensor_tensor(
                  g3.rearrange("p (t e) -> p t e", e=KNN),
                  w3.rearrange("p (t e) -> p t e", e=KNN),
                  wnrm[:, b0:b0 + bn].rearrange(
                      "p (t o) -> p t o", o=1).to_broadcast([P, bn, KNN]),
                  op=Alu.mult)

              # gather + weighted combine for this block
              for t in range(b0, b0 + bn):
                gts = []
                for k in range(KNN):
                    gt = gpool.tile([P, D2], f32, tag=f"g{k}")
                    nc.gpsimd.indirect_dma_start(
                        out=gt[:], out_offset=None,
                        in_=tsrc[s][:, :],
                        in_offset=IndirectOffsetOnAxis(
                            ap=idxb[:, t * 8 + k:t * 8 + k + 1], axis=0))
                    gts.append(gt)
                ra = rpool.tile([P, D2], f32, tag="ra")
                rb = rpool.tile([P, D2], f32, tag="rb")
                nc.vector.tensor_scalar_mul(ra[:], gts[0][:],
                                            wgt[:, t * 3:t * 3 + 1])
                nc.vector.scalar_tensor_tensor(
                    rb[:], gts[1][:], wgt[:, t * 3 + 1:t * 3 + 2], ra[:],
                    op0=Alu.mult, op1=Alu.add)
                nc.vector.scalar_tensor_tensor(
                    ra[:], gts[2][:], wgt[:, t * 3 + 2:t * 3 + 3], rb[:],
                    op0=Alu.mult, op1=Alu.add)

                if s < 3:
                    nc.sync.dma_start(
                        tsrc[s + 1][t * P:(t + 1) * P, st["cx"]:st["cx"] + D2],
                        ra[:])
                else:
                    # transpose [128, D2] to column layout and DMA to oi
                    dchunks = D2 // P
                    colb = rpool.tile([P, D2], f32, tag="colb")
                    for dd in range(dchunks):
                        pst = ps_tp.tile([P, P], f32, tag="pst")
                        nc.tensor.transpose(
                            pst[:], ra[:, dd * P:(dd + 1) * P], ident[:])
                        nc.scalar.copy(colb[:, dd * P:(dd + 1) * P], pst[:])
                    nc.sync.dma_start(
                        oi.rearrange("(dd p) (t j) -> p dd t j",
                                     p=P, j=P)[:, :, t, :],
                        colb[:].rearrange("p (dd j) -> p dd j", j=P))
    if split_waits:
        _split_multi_waits(nc)
    return nc


def _split_multi_waits(nc):
    """This walrus build rejects instructions carrying more than one sync
    wait. Hoist extra waits into same-engine NoOps inserted just before."""
    import concourse.mybir as mybir

    n = 0
    for f in nc.m.functions:
        for bb in f.blocks:
            il = bb.instructions
            i = 0
            while i < len(il):
                inst = il[i]
                si = getattr(inst, "sync_info", None)
                ow = list(si.on_wait) if si is not None else []
                if len(ow) > 1:
                    for w in ow[:-1]:
                        nop = mybir.InstNoOp(name=f"W{n}-{inst.name}",
                                             ins=[], outs=[])
                        n += 1
                        nop.engine = inst.engine
                        nop.sync_info = mybir.SyncInfo(on_update=[],
                                                       on_wait=[w])
                        il.insert(i, nop)
                        i += 1
                    inst.sync_info = mybir.SyncInfo(
                        on_update=list(si.on_update), on_wait=[ow[-1]])
                i += 1


def _patch_tile_drain():
    """This walrus build rejects >1 sync-wait on the kernel-tail Drain; spread
    the waits across single-wait SP nops instead."""
    import concourse.mybir as mybir
    import concourse.tile as tile
    from concourse.vector_clock import ScopedClock

    if getattr(tile.TileContext, "_drain_patched", False):
        return

    def _patched(self, tick_clock, wait_clock):
        nc = self.nc
        probe = nc.sync.nop()
        wait_clock.add_sem_waits(probe.ins,
                                 ScopedClock({None: tick_clock.global_clock}))
        si = probe.ins.sync_info
        ow = list(si.on_wait) if si is not None else []
        if len(ow) > 1:
            for w in ow[1:]:
                n2 = nc.sync.nop()
                n2.ins.sync_info = mybir.SyncInfo(on_update=[], on_wait=[w])
            probe.ins.sync_info = mybir.SyncInfo(on_update=list(si.on_update),
                                                 on_wait=[ow[0]])
        nc.sync.drain()
        nc.all_engine_barrier()
        assert self.sems is not None
        popped = nc._tile_sem_poison_stack.pop()
        assert popped is self._sem_poison
        nc.clear_and_free_semaphores(list(self.sems.allocated().values()))
        nc.all_engine_barrier()

    tile.TileContext._drain_and_barrier = _patched
    tile.TileContext._drain_patched = True


def _get_program(ns, cs, n_half):
    key = (tuple(ns), tuple(cs), n_half)
    if _CACHED["key"] != key:
        _CACHED["nc"] = _build_program(ns, cs, n_half)
        _CACHED["key"] = key
    return _CACHED["nc"]


def make_core_inputs(inputs, ns, n_half, core):
    """Slice/transform full inputs for one core (b = core//2, half = core%2)."""
    b, h = core // 2, core % 2
    d = {}
    x0h = np.ascontiguousarray(np.asarray(inputs["xyz0"])[b, h * n_half:(h + 1) * n_half])
    d["xt0"] = np.ascontiguousarray(x0h.T)
    d["xr0"] = x0h
    for i in range(1, 5):
        xi = np.ascontiguousarray(np.asarray(inputs[f"xyz{i}"])[b])
        d[f"xt{i}"] = np.ascontiguousarray(xi.T)
        d[f"xr{i}"] = xi
        d[f"f{i}"] = np.ascontiguousarray(np.asarray(inputs[f"x{i}"])[b])
    return d


def kernel(**inputs):
    from concourse.bass_utils import run_bass_kernel_spmd

    ns, cs = NS, CS
    n_half = ns[0] // 2
    nc = _get_program(ns, cs, n_half)

    in_maps = [make_core_inputs(inputs, ns, n_half, c) for c in range(8)]
    res = run_bass_kernel_spmd(nc, in_maps, core_ids=list(range(8)))

    dout = sum(cs)
    out = np.empty((B, dout, ns[0]), np.float32)
    out[:, :cs[0], :] = np.asarray(inputs["x0"])
    for c in range(8):
        b, h = c // 2, c % 2
        out[b, cs[0]:, h * n_half:(h + 1) * n_half] = res.results[c]["oi"]
    return out


# revision 20
# speedup vs baseline: 1.1402x; 1.0017x over previous
"""Trainium2 Bass kernel for a PointNet++-style feature-propagation decoder
(4 stages of kNN(k=3) inverse-distance-weighted feature interpolation).

Sharding: batch b = core//2 (data parallel over B=4), and the finest stage's
8192 query points split in half across each core pair (point parallel along N
per the sharding hint). Stages 0-2 are duplicated within a pair (cheap);
stage 3 dominates and is n-split. Output rows 0:64 are the x0 passthrough,
assembled on the host.

Per-core device pipeline per stage:
  - negated squared distances via one K=5 PE matmul per 128-query tile:
      A = [ax, ay, az, -|a|^2, -1], B = [2bx, 2by, 2bz, 1, |b|^2], A.B = -dist
  - top-3 neighbors via DVE max (top-8) + max_index
  - inverse-distance weights on DVE
  - feature gather via SWDGE indirect DMA (row gather from a DRAM source
    table, one row per partition), weighted 3-way combine via
    scalar_tensor_tensor (per-partition scalar FMA)
  - stage output rows are DMA'd into the next stage's DRAM source table;
    the final stage is transposed back to [D, N] layout via PE transposes.
"""

import numpy as np

P = 128
KNN = 3
EPS = 1e-8

B = 4
NS = [8192, 2048, 512, 128, 32]  # points per level, finest -> coarsest
CS = [64, 128, 256, 512, 1024]   # feature channels per level

_CACHED = {"nc": None, "key": None}


def _build_program(ns, cs, n_half, split_waits=True):
    """Trace the per-core Bass program. ns/cs as in reference (finest first).
    n_half: number of finest-level query points this core handles."""
    import contextlib

    import concourse.bass as bass
    import concourse.mybir as mybir
    import concourse.tile as tile
    from concourse.bass import IndirectOffsetOnAxis
    from concourse.masks import make_identity

    _patch_tile_drain()

    f32 = mybir.dt.float32
    u32 = mybir.dt.uint32
    Alu = mybir.AluOpType
    Axis = mybir.AxisListType

    # stage s: fine level (3-s) [half of level 0 for s=3], coarse level (4-s)
    # d2[s]: width of the source table gathered at stage s
    d2 = [cs[4]]
    for s in range(1, 4):
        d2.append(cs[4 - s] + d2[s - 1])
    stages = []
    for s in range(4):
        nf = ns[3 - s] if s < 3 else n_half
        stages.append(dict(nf=nf, S=ns[4 - s], d2=d2[s],
                           cx=(cs[3 - s] if s < 3 else None)))

    nc = bass.Bass("TRN2")

    # ---- external inputs (per core) ----
    xt = {}   # xyz transposed [3, N]
    xr = {}   # xyz rows [N, 3]
    for i, n in enumerate(ns):
        nn_ = n_half if i == 0 else n
        xt[i] = nc.dram_tensor(f"xt{i}", [3, nn_], f32, kind="ExternalInput")
        xr[i] = nc.dram_tensor(f"xr{i}", [nn_, 3], f32, kind="ExternalInput")
    feat = {}
    for i in range(1, 5):
        feat[i] = nc.dram_tensor(f"f{i}", [cs[i], ns[i]], f32, kind="ExternalInput")

    # ---- external output: interp part of final stage, [d2[3], n_half] ----
    oi = nc.dram_tensor("oi", [d2[3], n_half], f32, kind="ExternalOutput")

    # ---- internal DRAM source tables ----
    tsrc = []
    for s in range(4):
        tsrc.append(nc.dram_tensor(f"tsrc{s}", [stages[s]["S"], stages[s]["d2"]], f32))
    # scratch for reshaping row-layout norms into free-layout matmul rows
    nscr = {s: (nc.dram_tensor(f"nscrf{s}", [stages[s]["nf"]], f32),
                nc.dram_tensor(f"nscrc{s}", [max(stages[s]["S"], P)], f32))
            for s in range(4)}

    with tile.TileContext(nc) as tc, contextlib.ExitStack() as ctx:
        cpool = ctx.enter_context(tc.tile_pool(name="const", bufs=1))
        inpool = ctx.enter_context(tc.tile_pool(name="in", bufs=1))
        abpool = ctx.enter_context(tc.tile_pool(name="ab", bufs=1))
        ndpool = ctx.enter_context(tc.tile_pool(name="nd", bufs=3))
        smpool = ctx.enter_context(tc.tile_pool(name="sm", bufs=2))
        gpool = ctx.enter_context(tc.tile_pool(name="g", bufs=2))
        rpool = ctx.enter_context(tc.tile_pool(name="r", bufs=2))
        xtp = ctx.enter_context(tc.tile_pool(name="xtp", bufs=2))
        ps_nd = ctx.enter_context(tc.tile_pool(name="ps_nd", bufs=2, space="PSUM"))
        ps_tp = ctx.enter_context(tc.tile_pool(name="ps_tp", bufs=4, space="PSUM"))

        ident = cpool.tile([P, P], f32, tag="ident")
        make_identity(nc, ident[:])
        # constant rows (engines can't start at odd partitions; build at
        # partition 0 and DMA into place)
        maxn = max(st["nf"] for st in stages)
        cones = cpool.tile([1, maxn], f32, tag="cones")
        nc.vector.memset(cones[:], 1.0)
        cneg = cpool.tile([1, maxn], f32, tag="cneg")
        nc.vector.memset(cneg[:], -1.0)

        # ---------- build x^T parts of the source tables ----------
        def xt_into_table(fi, table, nrows, ncols):
            """write feat[fi]^T ([nrows points, ncols feats]) into
            table[:, 0:ncols] via PE transposes."""
            cchunks = (ncols + P - 1) // P
            rchunks = (nrows + P - 1) // P
            fsb = inpool.tile([min(ncols, P), cchunks * nrows], f32,
                              tag="fsb")
            src = feat[fi][:, :]
            if ncols > P:
                nc.sync.dma_start(
                    fsb[:].rearrange("p (cc n) -> p cc n", cc=cchunks),
                    src.rearrange("(cc p) n -> p cc n", p=P))
            else:
                nc.sync.dma_start(fsb[:ncols, :nrows], src)
            for t in range(rchunks):
                rows = min(P, nrows - t * P)
                xtt = xtp.tile([P, cchunks * P], f32, tag="xtt")
                for cc in range(cchunks):
                    c0 = cc * P
                    cw = min(P, ncols - c0)
                    pst = ps_tp.tile([P, P], f32, tag="pst")
                    nc.tensor.transpose(
                        pst[:rows, :cw],
                        fsb[:cw, cc * nrows + t * P: cc * nrows + t * P + rows],
                        ident[:])
                    nc.scalar.copy(xtt[:rows, c0:c0 + cw], pst[:rows, :cw])
                nc.sync.dma_start(table[t * P:t * P + rows, 0:ncols],
                                  xtt[:rows, :ncols])

        xt_into_table(4, tsrc[0], stages[0]["S"], cs[4])   # x4^T -> T0
        xt_into_table(3, tsrc[1], stages[1]["S"], cs[3])   # x3^T -> T1
        xt_into_table(2, tsrc[2], stages[2]["S"], cs[2])   # x2^T -> T2
        xt_into_table(1, tsrc[3], stages[3]["S"], cs[1])   # x1^T -> T3

        # ---------- row-layout squared norms ----------
        def row_norms(xr_dram, n, negate, tag):
            """[P, T] tile holding (+-)|p|^2 of n points, point (t*128+p) at
            [p, t] (or [p, 0] for p < n when n < 128)."""
            if n >= P:
                T = n // P
                rx = smpool.tile([P, T * 3], f32, tag=f"rx{tag}")
                nc.sync.dma_start(
                    rx[:].rearrange("p (t c) -> p t c", c=3),
                    xr_dram[:, :].rearrange("(t p) c -> p t c", p=P))
                pdim = P
            else:
                T = 1
                rx = smpool.tile([P, 3], f32, tag=f"rx{tag}")
                nc.sync.dma_start(rx[:n, :], xr_dram[:, :])
                pdim = n
            sq = smpool.tile([P, T * 3], f32, tag=f"sq{tag}")
            nc.vector.tensor_tensor(sq[:pdim, :T * 3], rx[:pdim, :T * 3],
                                    rx[:pdim, :T * 3], op=Alu.mult)
            nrm = smpool.tile([P, T], f32, tag=f"nrm{tag}")
            nc.vector.tensor_reduce(
                nrm[:pdim, :T],
                sq[:pdim, :T * 3].rearrange("p (t c) -> p t c", c=3),
                axis=Axis.X, op=Alu.add)
            if negate:
                nc.vector.tensor_scalar_mul(nrm[:pdim, :T], nrm[:pdim, :T], -1.0)
            return nrm, T

        # ---------- stages ----------
        for s, st in enumerate(stages):
            nf, S, D2 = st["nf"], st["S"], st["d2"]
            T = nf // P
            fine_lvl = 3 - s if s < 3 else 0
            coarse_lvl = 4 - s

            # A5 [5, nf]: rows 0-2 xt_fine, row3 -|a|^2, row4 -1
            a5 = abpool.tile([5, nf], f32, tag="a5")
            nc.sync.dma_start(a5[0:3, :], xt[fine_lvl][:, :])
            nfin, Tf = row_norms(xr[fine_lvl], nf, negate=True, tag="f")
            nc.sync.dma_start(
                nscr[s][0][:].rearrange("(t j) -> j t", j=P), nfin[:, :Tf])
            nc.sync.dma_start(a5[3:4, :], nscr[s][0][:])
            nc.sync.dma_start(a5[4:5, :], cneg[:, :nf])

            # B5 [5, S]: rows 0-2 2*xt_coarse, row3 1, row4 +|b|^2
            b5 = abpool.tile([5, S], f32, tag="b5")
            nc.sync.dma_start(b5[0:3, :], xt[coarse_lvl][:, :])
            nc.vector.tensor_scalar_mul(b5[0:3, :], b5[0:3, :], 2.0)
            nc.sync.dma_start(b5[3:4, :], cones[:, :S])
            ncr, Tc = row_norms(xr[coarse_lvl], S, negate=False, tag="c")
            if S >= P:
                nc.sync.dma_start(
                    nscr[s][1][:S].rearrange("(t j) -> j t", j=P),
                    ncr[:, :Tc])
                nc.sync.dma_start(b5[4:5, :], nscr[s][1][:S])
            else:
                nc.sync.dma_start(nscr[s][1][:S], ncr[:S, :1])
                nc.sync.dma_start(b5[4:5, :], nscr[s][1][:S])

            # ---- block-pipelined dist+top3 / weights / gather+combine ----
            maxb = smpool.tile([P, T * 8], f32, tag=f"maxb{s}")
            idxb = smpool.tile([P, T * 8], u32, tag=f"idxb{s}")
            dbuf = smpool.tile([P, T * KNN], f32, tag=f"dbuf{s}")
            wraw = smpool.tile([P, T * KNN], f32, tag=f"wraw{s}")
            wsum = smpool.tile([P, T], f32, tag=f"wsum{s}")
            wnrm = smpool.tile([P, T], f32, tag=f"wnrm{s}")
            wgt = smpool.tile([P, T * KNN], f32, tag=f"wgt{s}")
            nchunk = (S + 511) // 512
            BLK = 8
            for b0 in range(0, T, BLK):
              bn = min(BLK, T - b0)
              for t in range(b0, b0 + bn):
                nd_sb = ndpool.tile([P, S], f32, tag="nd_sb")
                pnd = ps_nd.tile([P, min(S, 1024)], f32, tag="pnd")
                for c in range(nchunk):
                    w = min(512, S - c * 512)
                    nc.tensor.matmul(
                        pnd[:, (c % 2) * 512:(c % 2) * 512 + w],
                        a5[:, t * P:(t + 1) * P],
                        b5[:, c * 512:c * 512 + w],
                        start=True, stop=True)
                    if c % 2 == 1 or c == nchunk - 1:
                        lo = (c // 2) * 1024
                        w2 = min(1024, S - lo)
                        nc.scalar.copy(nd_sb[:, lo:lo + w2], pnd[:, :w2])
                        if c != nchunk - 1:
                            pnd = ps_nd.tile([P, min(S, 1024)], f32, tag="pnd")
                nc.vector.max(maxb[:, t * 8:(t + 1) * 8], nd_sb[:])
                nc.vector.max_index(idxb[:, t * 8:(t + 1) * 8],
                                    maxb[:, t * 8:(t + 1) * 8], nd_sb[:])

              # weights for this block
              top3 = maxb[:, b0 * 8:(b0 + bn) * 8].rearrange(
                  "p (t e) -> p t e", e=8)[:, :, 0:KNN]
              d3 = dbuf[:, b0 * KNN:(b0 + bn) * KNN]
              w3 = wraw[:, b0 * KNN:(b0 + bn) * KNN]
              g3 = wgt[:, b0 * KNN:(b0 + bn) * KNN]
              nc.vector.tensor_scalar(d3.rearrange("p (t e) -> p t e", e=KNN),
                                      top3, -1.0, EPS, op0=Alu.mult, op1=Alu.add)
              nc.vector.reciprocal(w3, d3)
              nc.vector.tensor_reduce(
                  wsum[:, b0:b0 + bn], w3.rearrange("p (t e) -> p t e", e=KNN),
                  axis=Axis.X, op=Alu.add)
              nc.vector.reciprocal(wnrm[:, b0:b0 + bn], wsum[:, b0:b0 + bn])
              nc.vector.tensor_tensor(
                  g3.rearrange("p (t e) -> p t e", e=KNN),
                  w3.rearrange("p (t e) -> p t e", e=KNN),
                  wnrm[:, b0:b0 + bn].rearrange(
                      "p (t o) -> p t o", o=1).to_broadcast([P, bn, KNN]),
                  op=Alu.mult)

              # gather + weighted combine for this block
              for t in range(b0, b0 + bn):
                gts = []
                for k in range(KNN):
                    gt = gpool.tile([P, D2], f32, tag=f"g{k}")
                    nc.gpsimd.indirect_dma_start(
                        out=gt[:], out_offset=None,
                        in_=tsrc[s][:, :],
                        in_offset=IndirectOffsetOnAxis(
                            ap=idxb[:, t * 8 + k:t * 8 + k + 1], axis=0))
                    gts.append(gt)
                ra = rpool.tile([P, D2], f32, tag="ra")
                rb = rpool.tile([P, D2], f32, tag="rb")
                nc.scalar.activation(ra[:], gts[0][:],
                                     mybir.ActivationFunctionType.Identity,
                                     scale=wgt[:, t * 3:t * 3 + 1])
                nc.vector.scalar_tensor_tensor(
                    rb[:], gts[1][:], wgt[:, t * 3 + 1:t * 3 + 2], ra[:],
                    op0=Alu.mult, op1=Alu.add)
                nc.vector.scalar_tensor_tensor(
                    ra[:], gts[2][:], wgt[:, t * 3 + 2:t * 3 + 3], rb[:],
                    op0=Alu.mult, op1=Alu.add)

                if s < 3:
                    nc.sync.dma_start(
                        tsrc[s + 1][t * P:(t + 1) * P, st["cx"]:st["cx"] + D2],
                        ra[:])
                else:
                    # transpose [128, D2] to column layout and DMA to oi
                    dchunks = D2 // P
                    colb = rpool.tile([P, D2], f32, tag="colb")
                    for dd in range(dchunks):
                        pst = ps_tp.tile([P, P], f32, tag="pst")
                        nc.tensor.transpose(
                            pst[:], ra[:, dd * P:(dd + 1) * P], ident[:])
                        nc.scalar.copy(colb[:, dd * P:(dd + 1) * P], pst[:])
                    nc.sync.dma_start(
                        oi.rearrange("(dd p) (t j) -> p dd t j",
                                     p=P, j=P)[:, :, t, :],
                        colb[:].rearrange("p (dd j) -> p dd j", j=P))
    if split_waits:
        _split_multi_waits(nc)
    return nc


def _split_multi_waits(nc):
    """This walrus build rejects instructions carrying more than one sync
    wait. Hoist extra waits into same-engine NoOps inserted just before."""
    import concourse.mybir as mybir

    n = 0
    for f in nc.m.functions:
        for bb in f.blocks:
            il = bb.instructions
            i = 0
            while i < len(il):
                inst = il[i]
                si = getattr(inst, "sync_info", None)
                ow = list(si.on_wait) if si is not None else []
                if len(ow) > 1:
                    for w in ow[:-1]:
                        nop = mybir.InstNoOp(name=f"W{n}-{inst.name}",
                                             ins=[], outs=[])
                        n += 1
                        nop.engine = inst.engine
                        nop.sync_info = mybir.SyncInfo(on_update=[],
                                                       on_wait=[w])
                        il.insert(i, nop)
                        i += 1
                    inst.sync_info = mybir.SyncInfo(
                        on_update=list(si.on_update), on_wait=[ow[-1]])
                i += 1


def _patch_tile_drain():
    """This walrus build rejects >1 sync-wait on the kernel-tail Drain; spread
    the waits across single-wait SP nops instead."""
    import concourse.mybir as mybir
    import concourse.tile as tile
    from concourse.vector_clock import ScopedClock

    if getattr(tile.TileContext, "_drain_patched", False):
        return

    def _patched(self, tick_clock, wait_clock):
        nc = self.nc
        probe = nc.sync.nop()
        wait_clock.add_sem_waits(probe.ins,
                                 ScopedClock({None: tick_clock.global_clock}))
        si = probe.ins.sync_info
        ow = list(si.on_wait) if si is not None else []
        if len(ow) > 1:
            for w in ow[1:]:
                n2 = nc.sync.nop()
                n2.ins.sync_info = mybir.SyncInfo(on_update=[], on_wait=[w])
            probe.ins.sync_info = mybir.SyncInfo(on_update=list(si.on_update),
                                                 on_wait=[ow[0]])
        nc.sync.drain()
        nc.all_engine_barrier()
        assert self.sems is not None
        popped = nc._tile_sem_poison_stack.pop()
        assert popped is self._sem_poison
        nc.clear_and_free_semaphores(list(self.sems.allocated().values()))
        nc.all_engine_barrier()

    tile.TileContext._drain_and_barrier = _patched
    tile.TileContext._drain_patched = True


def _get_program(ns, cs, n_half):
    key = (tuple(ns), tuple(cs), n_half)
    if _CACHED["key"] != key:
        _CACHED["nc"] = _build_program(ns, cs, n_half)
        _CACHED["key"] = key
    return _CACHED["nc"]


def make_core_inputs(inputs, ns, n_half, core):
    """Slice/transform full inputs for one core (b = core//2, half = core%2)."""
    b, h = core // 2, core % 2
    d = {}
    x0h = np.ascontiguousarray(np.asarray(inputs["xyz0"])[b, h * n_half:(h + 1) * n_half])
    d["xt0"] = np.ascontiguousarray(x0h.T)
    d["xr0"] = x0h
    for i in range(1, 5):
        xi = np.ascontiguousarray(np.asarray(inputs[f"xyz{i}"])[b])
        d[f"xt{i}"] = np.ascontiguousarray(xi.T)
        d[f"xr{i}"] = xi
        d[f"f{i}"] = np.ascontiguousarray(np.asarray(inputs[f"x{i}"])[b])
    return d


def kernel(**inputs):
    from concourse.bass_utils import run_bass_kernel_spmd

    ns, cs = NS, CS
    n_half = ns[0] // 2
    nc = _get_program(ns, cs, n_half)

    in_maps = [make_core_inputs(inputs, ns, n_half, c) for c in range(8)]
    res = run_bass_kernel_spmd(nc, in_maps, core_ids=list(range(8)))

    dout = sum(cs)
    out = np.empty((B, dout, ns[0]), np.float32)
    out[:, :cs[0], :] = np.asarray(inputs["x0"])
    for c in range(8):
        b, h = c // 2, c % 2
        out[b, cs[0]:, h * n_half:(h + 1) * n_half] = res.results[c]["oi"]
    return out
